# revision 22
# baseline (speedup 1.0000x reference)
"""DiscreteWaveletUpsample Trainium2 kernel.

Math: out = conv3x3(haar_upsample(conv3x3(x, pre_w) + pre_b), post_w) + post_b

Device algorithm (per core, one batch sample, data-parallel over batch=8):

  * The fixed Haar reconstruction is folded into the pre-conv weights:
    Y(p,q)[c,h,w] (polyphase components of the upsampled image) is a 3x3
    conv of x with effective weights Weff[p,q,c].

  * x is stored flat [128, 132*W] bf16 (host-padded: two zero rows above
    and below, duplicated on both partition halves) so every tap's rhs
    is ONE contiguous 512-element window: full-width row slices are
    contiguous, and the column offset kx-1 just shifts the window.  The
    column wrap-around (row r col -1 = row r-1 col 127) contaminates
    only Y columns 0 and 127; a small repair pass of column-GEMMs
    against host-prepared edge columns (xe) overwrites them correctly.

  * Stage 1 (per 4-row tile, out ctile p): 9 tap-matmuls K=64 accumulate
    [Y(p,0); Y(p,1)] (M=128) in PSUM; 64x128 row-tiled mode with row
    group = tile parity runs two tiles' streams concurrently.
    Evacuation (ScalarE/VectorE alternating) adds bias, writes bf16 Y
    into zero-bordered [130,130] images: partitions q*64+c = Y(p,q)[c].

  * Stage 2 = the post conv in polyphase space, FOUR concurrent streams:
    row group = q_in (the input component's native half -- for every tap
    the two out-components map to distinct q_in, so no partition-swapped
    Y duplicate is needed), col group = tile parity across a pair of
    tiles.  Tap order starts AND stops each accumulator in its own
    quadrant.  p=0 banks evacuate under the p=1 matmuls.

  * Output staged bf16 (halves HBM traffic, 2x DVE rate), host upcast.
"""

import numpy as np
import ml_dtypes

import concourse.bass as bass
import concourse.mybir as mybir
import concourse.tile as tile
from concourse import bacc
from concourse.tile_rust import add_dep_helper
from concourse.bass_utils import run_bass_kernel_spmd

N_CORES = 8

C = 64
H = W = 128
HP = H + 4      # two zero rows above + below (x); ybuf uses 130x130
# stage-1 tap order (any order valid; center first).
TAPS1 = [(1, 1), (0, 0), (0, 1), (0, 2), (1, 0),
         (1, 2), (2, 0), (2, 1), (2, 2)]
# stage-2 tap order: kx=1 taps first/last so every accumulator's
# start and stop land in its own PE quadrant (q_in == q when kx == 1).
TAPS2 = [(1, 1), (0, 0), (0, 2), (2, 0), (2, 2),
         (1, 0), (1, 2), (0, 1), (2, 1)]

F32 = mybir.dt.float32
BF16 = mybir.dt.bfloat16
NP_BF16 = ml_dtypes.bfloat16
IDENT = mybir.ActivationFunctionType.Identity


# ----------------------------------------------------------------------------
# Host-side weight preparation
# ----------------------------------------------------------------------------

def _build_stage1_weights(pre_w, pre_b):
    """w1[p, ky, kx, cin, m] float32 (m = q*64 + c), b1[m, p]."""
    lo = np.array([0.5, 0.5], np.float32)
    hi = np.array([0.5, -0.5], np.float32)
    filt = np.stack([np.outer(lo, lo), np.outer(lo, hi),
                     np.outer(hi, lo), np.outer(hi, hi)], axis=0)
    pw = pre_w.reshape(4, C, C, 3, 3).astype(np.float32)
    pb = pre_b.reshape(4, C).astype(np.float32)
    weff = np.einsum('spq,scikl->pqcikl', filt, pw)
    beff = np.einsum('spq,sc->pqc', filt, pb)
    w1 = np.transpose(weff, (0, 4, 5, 3, 1, 2)).reshape(2, 3, 3, C, 2 * C)
    b1 = beff.reshape(2, 2 * C).T.copy()
    return w1, b1


def _tap_decomp(p, q, ky, kx):
    jy = p + ky - 1
    p_in = jy & 1
    dy = (jy - p_in) >> 1
    jx = q + kx - 1
    q_in = jx & 1
    dx = (jx - q_in) >> 1
    return p_in, q_in, dy, dx


def _build_stage2_weights(post_w):
    """w2[128, 9*64]: block (ky*3+kx) = post_w[:,:,ky,kx].T, both halves."""
    w2 = np.zeros((2 * C, 9 * C), np.float32)
    pwf = post_w.astype(np.float32)
    for ky in range(3):
        for kx in range(3):
            blk = (ky * 3 + kx) * C
            w2[0:C, blk:blk + C] = pwf[:, :, ky, kx].T
            w2[C:2 * C, blk:blk + C] = pwf[:, :, ky, kx].T
    return w2


# ----------------------------------------------------------------------------
# Device module
# ----------------------------------------------------------------------------

def _build_module():
    nc = bacc.Bacc("TRN2", target_bir_lowering=False, debug=False,
                   num_devices=N_CORES)

    # xpad: [C, 132*W] flat, zero rows 0,1,130,131 host-baked.
    x_d = nc.dram_tensor("xpad", [C, HP * W], BF16, kind="ExternalInput")
    # xe: [C, 132*4] edge columns (x cols 0,1,126,127), zero rows baked.
    xe_d = nc.dram_tensor("xe", [C, HP * 4], BF16, kind="ExternalInput")
    w1_d = nc.dram_tensor("w1", [128, 18 * 128], BF16, kind="ExternalInput")
    # w1 with the m-halves of every idx block swapped (repair pass writes
    # the partition-swapped Y duplicate natively with these).
    w1x_d = nc.dram_tensor("w1x", [128, 18 * 128], BF16,
                           kind="ExternalInput")
    # b1 cols 0,1 = bias[m, p]; cols 2,3 = half-swapped (for the j=1
    # repair accumulators whose partitions hold swapped components).
    b1_d = nc.dram_tensor("b1", [128, 4], F32, kind="ExternalInput")
    w2_d = nc.dram_tensor("w2", [128, 9 * C], BF16, kind="ExternalInput")
    b2_d = nc.dram_tensor("b2", [128, 1], F32, kind="ExternalInput")
    out_d = nc.dram_tensor("out", [C, 2 * H, 2 * W], BF16,
                           kind="ExternalOutput")

    with tile.TileContext(nc) as tc:
        with (
            tc.tile_pool(name="const", bufs=1) as const,
            tc.tile_pool(name="xbuf", bufs=1) as xpool,
            tc.tile_pool(name="ybuf", bufs=1) as ypool,
            tc.tile_pool(name="psum", bufs=8, space="PSUM") as psum_pool,
            tc.tile_pool(name="stage", bufs=4) as stg,
        ):
            # ---- constants (scalar=Activation DMA queue) ----
            # w1 split into tap-ordered pieces: supply must stay ahead of
            # the tap consumption rate (~1.8us per tap pair of columns).
            w1_s = const.tile([128, 18 * 128], BF16)
            nc.scalar.dma_start(out=w1_s[:, 1024:1280],
                                in_=w1_d[:, 1024:1280])
            nc.scalar.dma_start(out=w1_s[:, 0:1024], in_=w1_d[:, 0:1024])
            nc.scalar.dma_start(out=w1_s[:, 1280:], in_=w1_d[:, 1280:])
            b1_s = const.tile([128, 4], F32)
            nc.scalar.dma_start(out=b1_s[:], in_=b1_d[:])
            w2_s = const.tile([128, 9 * C], BF16)
            b2_s = const.tile([128, 1], F32)
            w1x_s = const.tile([128, 18 * 128], BF16)

            # Warm the ScalarE activation table under the input DMAs.
            warm = const.tile([128, 1], F32)
            nc.vector.memset(warm[:], 0.0)
            nc.scalar.activation(warm[:], warm[:], IDENT)

            # ---- x image, flat padded, duplicated on both halves ----
            xp = xpool.tile([128, HP * W], BF16)
            xe = xpool.tile([128, HP * 4], BF16)
            bounds = [0, 16, 48, 80, 112, HP]
            for r0, r1 in zip(bounds[:-1], bounds[1:]):
                for g, eng in ((0, nc.sync), (1, nc.gpsimd)):
                    eng.dma_start(
                        out=xp[g * C:(g + 1) * C, r0 * W:r1 * W],
                        in_=x_d[:, r0 * W:r1 * W])
            for g, eng in ((0, nc.sync), (1, nc.gpsimd)):
                eng.dma_start(out=xe[g * C:(g + 1) * C, :], in_=xe_d[:])
            nc.scalar.dma_start(out=w2_s[:], in_=w2_d[:])
            nc.scalar.dma_start(out=b2_s[:], in_=b2_d[:])
            nc.scalar.dma_start(out=w1x_s[:], in_=w1x_d[:])

            # ---- Y buffers [130,130], zero borders; ybufs[p][j]:
            # partitions 0-63 = Y(p,j), 64-127 = Y(p,1-j); j=0 written by
            # stage-1 evac, j=1 is the partition-swapped DMA copy (so
            # every component is available on both PE row-halves) ----
            ybufs = [[None, None], [None, None]]
            for p in (0, 1):
                for j in (0, 1):
                    yb = ypool.tile([128, H + 2, W + 2], BF16,
                                    name=f"ybuf{p}{j}")
                    ybufs[p][j] = yb
                    nc.gpsimd.memset(yb[:, 0, :], 0.0)
                    nc.gpsimd.memset(yb[:, H + 1, :], 0.0)
                    if j == 0:
                        # cols 0,129 = zero borders; cols 1,128 pre-zeroed
                        # so the early dup chunks (before the repair pass
                        # rewrites them) read initialized data.
                        nc.gpsimd.memset(yb[:, 1:H + 1, 0:2], 0.0)
                        nc.gpsimd.memset(yb[:, 1:H + 1, W:W + 2], 0.0)

            # ---- PE warm-up: the tensor engine would idle ~3us waiting
            # for w1/x; dummy matmuls keep it busy so the clock ramps to
            # the high p-state before the real stream starts ----
            dm = const.tile([128, 512], BF16)
            nc.vector.memset(dm[:], 0.0)

            # ---- global PE emission-order chain ----
            state = {"prev": None}

            def mm(out_ap, w_ap, rhs_ap, start, stop, pos):
                inst = nc.tensor.matmul(out_ap, w_ap, rhs_ap,
                                        start=start, stop=stop,
                                        tile_position=pos)
                if state["prev"] is not None:
                    add_dep_helper(inst.ins, state["prev"], sync=False,
                                   reason="pe-emission-order")
                state["prev"] = inst.ins

            for i in range(14):
                g = i % 2
                acc = psum_pool.tile([128, 4, W], F32, name="ps", tag="ps")
                mm(acc[:, :, :], dm[g * C:(g + 1) * C, 0:128],
                   dm[g * C:(g + 1) * C, :], True, True, (g * C, 0))

            def stage1_super(sup):
                ts_all = list(range(4 * sup, 4 * sup + 4))
                for p in (0, 1):
                    accs = {t: psum_pool.tile([128, 4, W], F32,
                                              name="ps", tag="ps")
                            for t in ts_all}
                    for k, (ky, kx) in enumerate(TAPS1):
                        idx = (ky * 3 + kx) * 2 + p
                        for g in (0, 1):
                            gs = slice(g * C, (g + 1) * C)
                            for t in ts_all[g::2]:
                                base = (4 * t + ky + 1) * W + kx - 1
                                mm(accs[t][:, :, :],
                                   w1_s[gs, idx * 128:(idx + 1) * 128],
                                   xp[gs, base:base + 4 * W],
                                   k == 0, k == 8, (g * C, 0))
                    for t in ts_all:
                        h0 = 4 * t
                        dst = ybufs[p][0][:, h0 + 1:h0 + 5, 1:W + 1]
                        if t % 2 == 0:
                            nc.scalar.activation(dst, accs[t][:, :, :], IDENT,
                                                 bias=b1_s[:, p:p + 1])
                        else:
                            nc.vector.tensor_scalar_add(dst, accs[t][:, :, :],
                                                        b1_s[:, p:p + 1])
                    # partition-swapped duplicate, one chunk per (p, half)
                    r0, r1 = 4 * ts_all[0] + 1, 4 * ts_all[-1] + 5
                    nc.sync.dma_start(
                        out=ybufs[p][1][0:C, r0:r1, :],
                        in_=ybufs[p][0][C:128, r0:r1, :])
                    nc.gpsimd.dma_start(
                        out=ybufs[p][1][C:128, r0:r1, :],
                        in_=ybufs[p][0][0:C, r0:r1, :])

            def stage1_repair():
                # Overwrite the wrap-contaminated Y columns 0 and 127 with
                # column-GEMMs against the xe edge columns.  j=1 accs use
                # the half-swapped weights (and swapped bias rows) so the
                # partition-swapped duplicate is written natively too.
                for p in (0, 1):
                    for e in (0, 1):
                        for j in (0, 1):
                            g = (2 * p + e + j) % 2
                            gs = slice(g * C, (g + 1) * C)
                            ws = w1_s if j == 0 else w1x_s
                            acc = psum_pool.tile([128, 4, W], F32,
                                                 name="ps", tag="ps")
                            kxs = (1, 2) if e == 0 else (0, 1)
                            taps = [(ky, kx) for kx in kxs
                                    for ky in (0, 1, 2)]
                            for k, (ky, kx) in enumerate(taps):
                                idx = (ky * 3 + kx) * 2 + p
                                # xe col: x col kx-1 -> 0,1; 126+kx -> 2,3
                                xc = (kx - 1) if e == 0 else (kx + 2)
                                rhs = xe[gs, (ky + 1) * 4 + xc:
                                         (ky + 129) * 4 + xc:4]
                                mm(acc[:, 0:1, :],
                                   ws[gs, idx * 128:(idx + 1) * 128],
                                   rhs, k == 0, k == len(taps) - 1,
                                   (g * C, 0))
                            bcol = 2 * j + p
                            dst = ybufs[p][j][:, 1:H + 1,
                                              e * 127 + 1:e * 127 + 2]
                            if (e + j) % 2 == 0:
                                nc.scalar.activation(
                                    dst, acc[:, 0:1, :], IDENT,
                                    bias=b1_s[:, bcol:bcol + 1])
                            else:
                                nc.vector.tensor_scalar_add(
                                    dst, acc[:, 0:1, :],
                                    b1_s[:, bcol:bcol + 1])

            def stage2_pair(j):
                st = stg.tile([128, 8, 2 * W], BF16, name="st", tag="st")
                accs = {}
                for pp in (0, 1):
                    for qq in (0, 1):
                        for c in (0, 1):
                            accs[pp, qq, c] = psum_pool.tile(
                                [128, 4, W], F32, name="ps", tag="ps")
                for pp in (0, 1):
                    for k, (ky, kx) in enumerate(TAPS2):
                        blk = (ky * 3 + kx) * C
                        for c in (0, 1):
                            h0 = 4 * (2 * j + c)
                            cs = slice(c * C, (c + 1) * C)
                            for qq in (0, 1):
                                # fixed row group g=qq per accumulator (a
                                # PSUM bank must only ever be written from
                                # ONE quadrant: quadrants run concurrently
                                # and same-bank RMW from two would race).
                                p_in, q_in, dy, dx = _tap_decomp(
                                    pp, qq, ky, kx)
                                gs = slice(qq * C, (qq + 1) * C)
                                rhs = ybufs[p_in][q_in ^ qq][
                                    gs, h0 + dy + 1:h0 + dy + 5,
                                    dx + 1:dx + 1 + W]
                                mm(accs[pp, qq, c][cs, :, :],
                                   w2_s[gs, blk:blk + C],
                                   rhs, k == 0, k == 8, (qq * C, c * C))
                    for i, (qq, c) in enumerate(
                            ((0, 0), (0, 1), (1, 0), (1, 1))):
                        cs = slice(c * C, (c + 1) * C)
                        dst = st[cs, pp::2, qq::2]
                        if (i + pp) % 2 == 0:
                            nc.scalar.activation(dst, accs[pp, qq, c][cs],
                                                 IDENT, bias=b2_s[cs, 0:1])
                        else:
                            nc.vector.tensor_scalar_add(
                                dst, accs[pp, qq, c][cs], b2_s[cs, 0:1])
                if j < H // 8 - 1:
                    for c, eng in ((0, nc.sync), (1, nc.gpsimd)):
                        t = 2 * j + c
                        cs = slice(c * C, (c + 1) * C)
                        eng.dma_start(out=out_d[:, 8 * t:8 * t + 8, :],
                                      in_=st[cs, :, :])
                else:
                    for c in (0, 1):
                        t = 2 * j + c
                        cs = slice(c * C, (c + 1) * C)
                        for h, eng in ((0, nc.sync), (4, nc.gpsimd)):
                            eng.dma_start(
                                out=out_d[:, 8 * t + h:8 * t + h + 4, :],
                                in_=st[cs, h:h + 4, :])

            for sup in range(H // 16):
                stage1_super(sup)
            stage1_repair()
            for j in range(H // 8):
                stage2_pair(j)

    nc.compile()
    return nc


_MODULE_CACHE = {}


def _get_module():
    if "nc" not in _MODULE_CACHE:
        _MODULE_CACHE["nc"] = _build_module()
    return _MODULE_CACHE["nc"]


# ----------------------------------------------------------------------------
# Entry point
# ----------------------------------------------------------------------------

def prep_weight_map(pre_w, pre_b, post_w, post_b):
    w1, b1 = _build_stage1_weights(np.asarray(pre_w), np.asarray(pre_b))
    w2 = _build_stage2_weights(np.asarray(post_w))
    b2 = np.asarray(post_b, np.float32).reshape(C, 1)

    w1_half = np.transpose(w1, (3, 1, 2, 0, 4)).reshape(C, 18 * 128)
    w1_flat = np.ascontiguousarray(
        np.concatenate([w1_half, w1_half], axis=0)).astype(NP_BF16)
    # half-swapped m columns per idx block + half-swapped bias rows, for
    # the repair accs that write the partition-swapped Y duplicate.
    w1x = w1_flat.reshape(128, 18, 2, 64)[:, :, ::-1, :].reshape(128, -1)
    b1sw = np.roll(b1, 64, axis=0)
    b1_4 = np.concatenate([b1, b1sw], axis=1)          # [128, 4]
    return {
        "w1": w1_flat,
        "w1x": np.ascontiguousarray(w1x),
        "b1": np.ascontiguousarray(b1_4, np.float32),
        "w2": np.ascontiguousarray(w2).astype(NP_BF16),
        "b2": np.ascontiguousarray(np.vstack([b2, b2]), np.float32),
    }


def run(x, pre_w, pre_b, post_w, post_b, trace=False):
    x = np.asarray(x, np.float32)
    B = x.shape[0]
    assert B == N_CORES and x.shape == (B, C, H, W)

    wmap = prep_weight_map(pre_w, pre_b, post_w, post_b)
    x_bf = x.astype(NP_BF16)

    in_maps = []
    for b in range(B):
        xpad = np.zeros((C, HP, W), NP_BF16)
        xpad[:, 2:H + 2, :] = x_bf[b]
        xe = np.zeros((C, HP, 4), NP_BF16)
        xe[:, 2:H + 2, 0:2] = x_bf[b][:, :, 0:2]
        xe[:, 2:H + 2, 2:4] = x_bf[b][:, :, 126:128]
        in_maps.append({
            "xpad": np.ascontiguousarray(xpad.reshape(C, HP * W)),
            "xe": np.ascontiguousarray(xe.reshape(C, HP * 4)),
            **wmap,
        })

    nc = _get_module()
    res = run_bass_kernel_spmd(nc, in_maps, core_ids=list(range(N_CORES)),
                               trace=trace)
    out = np.stack([res.results[b]["out"].astype(np.float32)
                    for b in range(B)])
    return out, res


def kernel(x, pre_w, pre_b, post_w, post_b):
    out, _ = run(x, pre_w, pre_b, post_w, post_b)
    return out


# revision 23
# speedup vs baseline: 1.0461x; 1.0461x over previous
"""DiscreteWaveletUpsample Trainium2 kernel.

Math: out = conv3x3(haar_upsample(conv3x3(x, pre_w) + pre_b), post_w) + post_b

Device algorithm (per core, one batch sample, data-parallel over batch=8):

  * The fixed Haar reconstruction is folded into the pre-conv weights:
    Y(p,q)[c,h,w] (polyphase components of the upsampled image) is a 3x3
    conv of x with effective weights Weff[p,q,c].

  * x is stored flat [128, 132*W] bf16 (host-padded: two zero rows above
    and below, duplicated on both partition halves) so every tap's rhs
    is ONE contiguous 512-element window: full-width row slices are
    contiguous, and the column offset kx-1 just shifts the window.  The
    column wrap-around (row r col -1 = row r-1 col 127) contaminates
    only Y columns 0 and 127; a small repair pass of column-GEMMs
    against host-prepared edge columns (xe) overwrites them correctly.

  * Stage 1 (per 4-row tile, out ctile p): 9 tap-matmuls K=64 accumulate
    [Y(p,0); Y(p,1)] (M=128) in PSUM; 64x128 row-tiled mode with row
    group = tile parity runs two tiles' streams concurrently.
    Evacuation (ScalarE/VectorE alternating) adds bias, writes bf16 Y
    into zero-bordered [130,130] images: partitions q*64+c = Y(p,q)[c].

  * Stage 2 = the post conv in polyphase space, FOUR concurrent streams:
    row group = q_in (the input component's native half -- for every tap
    the two out-components map to distinct q_in, so no partition-swapped
    Y duplicate is needed), col group = tile parity across a pair of
    tiles.  Tap order starts AND stops each accumulator in its own
    quadrant.  p=0 banks evacuate under the p=1 matmuls.

  * Output staged bf16 (halves HBM traffic, 2x DVE rate), host upcast.
"""

import numpy as np
import ml_dtypes

import concourse.bass as bass
import concourse.mybir as mybir
import concourse.tile as tile
from concourse import bacc
from concourse.tile_rust import add_dep_helper
from concourse.bass_utils import run_bass_kernel_spmd

N_CORES = 8

C = 64
H = W = 128
HP = H + 4      # two zero rows above + below (x); ybuf uses 130x130
# stage-1 tap order (any order valid; center first).
TAPS1 = [(1, 1), (0, 0), (0, 1), (0, 2), (1, 0),
         (1, 2), (2, 0), (2, 1), (2, 2)]
# stage-2 tap order: kx=1 taps first/last so every accumulator's
# start and stop land in its own PE quadrant (q_in == q when kx == 1).
TAPS2 = [(1, 1), (0, 0), (0, 2), (2, 0), (2, 2),
         (1, 0), (1, 2), (0, 1), (2, 1)]

F32 = mybir.dt.float32
BF16 = mybir.dt.bfloat16
NP_BF16 = ml_dtypes.bfloat16
IDENT = mybir.ActivationFunctionType.Identity


# ----------------------------------------------------------------------------
# Host-side weight preparation
# ----------------------------------------------------------------------------

def _build_stage1_weights(pre_w, pre_b):
    """w1[p, ky, kx, cin, m] float32 (m = q*64 + c), b1[m, p]."""
    lo = np.array([0.5, 0.5], np.float32)
    hi = np.array([0.5, -0.5], np.float32)
    filt = np.stack([np.outer(lo, lo), np.outer(lo, hi),
                     np.outer(hi, lo), np.outer(hi, hi)], axis=0)
    pw = pre_w.reshape(4, C, C, 3, 3).astype(np.float32)
    pb = pre_b.reshape(4, C).astype(np.float32)
    weff = np.einsum('spq,scikl->pqcikl', filt, pw)
    beff = np.einsum('spq,sc->pqc', filt, pb)
    w1 = np.transpose(weff, (0, 4, 5, 3, 1, 2)).reshape(2, 3, 3, C, 2 * C)
    b1 = beff.reshape(2, 2 * C).T.copy()
    return w1, b1


def _tap_decomp(p, q, ky, kx):
    jy = p + ky - 1
    p_in = jy & 1
    dy = (jy - p_in) >> 1
    jx = q + kx - 1
    q_in = jx & 1
    dx = (jx - q_in) >> 1
    return p_in, q_in, dy, dx


def _build_stage2_weights(post_w):
    """w2[128, 9*64]: block (ky*3+kx) = post_w[:,:,ky,kx].T, both halves."""
    w2 = np.zeros((2 * C, 9 * C), np.float32)
    pwf = post_w.astype(np.float32)
    for ky in range(3):
        for kx in range(3):
            blk = (ky * 3 + kx) * C
            w2[0:C, blk:blk + C] = pwf[:, :, ky, kx].T
            w2[C:2 * C, blk:blk + C] = pwf[:, :, ky, kx].T
    return w2


# ----------------------------------------------------------------------------
# Device module
# ----------------------------------------------------------------------------

def _build_module():
    nc = bacc.Bacc("TRN2", target_bir_lowering=False, debug=False,
                   num_devices=N_CORES)

    # xpad: [C, 132*W] flat, zero rows 0,1,130,131 host-baked.
    x_d = nc.dram_tensor("xpad", [C, HP * W], BF16, kind="ExternalInput")
    # xe: [C, 132*4] edge columns (x cols 0,1,126,127), zero rows baked.
    xe_d = nc.dram_tensor("xe", [C, HP * 4], BF16, kind="ExternalInput")
    w1_d = nc.dram_tensor("w1", [128, 18 * 128], BF16, kind="ExternalInput")
    # w1 with the m-halves of every idx block swapped (repair pass writes
    # the partition-swapped Y duplicate natively with these).
    w1x_d = nc.dram_tensor("w1x", [128, 18 * 128], BF16,
                           kind="ExternalInput")
    # b1 cols 0,1 = bias[m, p]; cols 2,3 = half-swapped (for the j=1
    # repair accumulators whose partitions hold swapped components).
    b1_d = nc.dram_tensor("b1", [128, 4], F32, kind="ExternalInput")
    w2_d = nc.dram_tensor("w2", [128, 9 * C], BF16, kind="ExternalInput")
    b2_d = nc.dram_tensor("b2", [128, 1], F32, kind="ExternalInput")
    out_d = nc.dram_tensor("out", [C, 2 * H, 2 * W], BF16,
                           kind="ExternalOutput")

    with tile.TileContext(nc) as tc:
        with (
            tc.tile_pool(name="const", bufs=1) as const,
            tc.tile_pool(name="xbuf", bufs=1) as xpool,
            tc.tile_pool(name="ybuf", bufs=1) as ypool,
            tc.tile_pool(name="psum", bufs=8, space="PSUM") as psum_pool,
            tc.tile_pool(name="stage", bufs=4) as stg,
        ):
            # ---- constants (scalar=Activation DMA queue) ----
            # w1 split into tap-ordered pieces: supply must stay ahead of
            # the tap consumption rate (~1.8us per tap pair of columns).
            w1_s = const.tile([128, 18 * 128], BF16)
            nc.scalar.dma_start(out=w1_s[:, 1024:1280],
                                in_=w1_d[:, 1024:1280])
            nc.scalar.dma_start(out=w1_s[:, 0:1024], in_=w1_d[:, 0:1024])
            nc.scalar.dma_start(out=w1_s[:, 1280:], in_=w1_d[:, 1280:])
            b1_s = const.tile([128, 4], F32)
            nc.scalar.dma_start(out=b1_s[:], in_=b1_d[:])
            w2_s = const.tile([128, 9 * C], BF16)
            b2_s = const.tile([128, 1], F32)
            w1x_s = const.tile([128, 18 * 128], BF16)

            # Warm the ScalarE activation table under the input DMAs.
            warm = const.tile([128, 1], F32)
            nc.vector.memset(warm[:], 0.0)
            nc.scalar.activation(warm[:], warm[:], IDENT)

            # ---- x image, flat padded, duplicated on both halves ----
            xp = xpool.tile([128, HP * W], BF16)
            xe = xpool.tile([128, HP * 4], BF16)
            bounds = list(range(0, HP, 16)) + [HP]
            for r0, r1 in zip(bounds[:-1], bounds[1:]):
                for g, eng in ((0, nc.sync), (1, nc.gpsimd)):
                    eng.dma_start(
                        out=xp[g * C:(g + 1) * C, r0 * W:r1 * W],
                        in_=x_d[:, r0 * W:r1 * W])
            for g, eng in ((0, nc.sync), (1, nc.gpsimd)):
                eng.dma_start(out=xe[g * C:(g + 1) * C, :], in_=xe_d[:])
            nc.scalar.dma_start(out=w2_s[:], in_=w2_d[:])
            nc.scalar.dma_start(out=b2_s[:], in_=b2_d[:])
            nc.scalar.dma_start(out=w1x_s[:], in_=w1x_d[:])

            # ---- Y buffers [130,130], zero borders; ybufs[p][j]:
            # partitions 0-63 = Y(p,j), 64-127 = Y(p,1-j); j=0 written by
            # stage-1 evac, j=1 is the partition-swapped DMA copy (so
            # every component is available on both PE row-halves) ----
            ybufs = [[None, None], [None, None]]
            for p in (0, 1):
                for j in (0, 1):
                    yb = ypool.tile([128, H + 2, W + 2], BF16,
                                    name=f"ybuf{p}{j}")
                    ybufs[p][j] = yb
                    nc.gpsimd.memset(yb[:, 0, :], 0.0)
                    nc.gpsimd.memset(yb[:, H + 1, :], 0.0)
                    if j == 0:
                        # cols 0,129 = zero borders; cols 1,128 pre-zeroed
                        # so the early dup chunks (before the repair pass
                        # rewrites them) read initialized data.
                        nc.gpsimd.memset(yb[:, 1:H + 1, 0:2], 0.0)
                        nc.gpsimd.memset(yb[:, 1:H + 1, W:W + 2], 0.0)

            # ---- PE warm-up: the tensor engine would idle ~3us waiting
            # for w1/x; dummy matmuls keep it busy so the clock ramps to
            # the high p-state before the real stream starts ----
            dm = const.tile([128, 512], BF16)
            nc.vector.memset(dm[:], 0.0)

            # ---- global PE emission-order chain ----
            state = {"prev": None}

            def mm(out_ap, w_ap, rhs_ap, start, stop, pos):
                inst = nc.tensor.matmul(out_ap, w_ap, rhs_ap,
                                        start=start, stop=stop,
                                        tile_position=pos)
                if state["prev"] is not None:
                    add_dep_helper(inst.ins, state["prev"], sync=False,
                                   reason="pe-emission-order")
                state["prev"] = inst.ins

            for i in range(14):
                g = i % 2
                acc = psum_pool.tile([128, 4, W], F32, name="ps", tag="ps")
                mm(acc[:, :, :], dm[g * C:(g + 1) * C, 0:128],
                   dm[g * C:(g + 1) * C, :], True, True, (g * C, 0))

            def stage1_super(sup):
                ts_all = list(range(4 * sup, 4 * sup + 4))
                for p in (0, 1):
                    accs = {t: psum_pool.tile([128, 4, W], F32,
                                              name="ps", tag="ps")
                            for t in ts_all}
                    for k, (ky, kx) in enumerate(TAPS1):
                        idx = (ky * 3 + kx) * 2 + p
                        for g in (0, 1):
                            gs = slice(g * C, (g + 1) * C)
                            for t in ts_all[g::2]:
                                base = (4 * t + ky + 1) * W + kx - 1
                                mm(accs[t][:, :, :],
                                   w1_s[gs, idx * 128:(idx + 1) * 128],
                                   xp[gs, base:base + 4 * W],
                                   k == 0, k == 8, (g * C, 0))
                    for t in ts_all:
                        h0 = 4 * t
                        dst = ybufs[p][0][:, h0 + 1:h0 + 5, 1:W + 1]
                        if t % 2 == 0:
                            nc.scalar.activation(dst, accs[t][:, :, :], IDENT,
                                                 bias=b1_s[:, p:p + 1])
                        else:
                            nc.vector.tensor_scalar_add(dst, accs[t][:, :, :],
                                                        b1_s[:, p:p + 1])
                    # partition-swapped duplicate, one chunk per (p, half)
                    r0, r1 = 4 * ts_all[0] + 1, 4 * ts_all[-1] + 5
                    nc.sync.dma_start(
                        out=ybufs[p][1][0:C, r0:r1, :],
                        in_=ybufs[p][0][C:128, r0:r1, :])
                    nc.gpsimd.dma_start(
                        out=ybufs[p][1][C:128, r0:r1, :],
                        in_=ybufs[p][0][0:C, r0:r1, :])

            def stage1_repair():
                # Overwrite the wrap-contaminated Y columns 0 and 127 with
                # column-GEMMs against the xe edge columns.  j=1 accs use
                # the half-swapped weights (and swapped bias rows) so the
                # partition-swapped duplicate is written natively too.
                for p in (0, 1):
                    for e in (0, 1):
                        for j in (0, 1):
                            g = (2 * p + e + j) % 2
                            gs = slice(g * C, (g + 1) * C)
                            ws = w1_s if j == 0 else w1x_s
                            acc = psum_pool.tile([128, 4, W], F32,
                                                 name="ps", tag="ps")
                            kxs = (1, 2) if e == 0 else (0, 1)
                            taps = [(ky, kx) for kx in kxs
                                    for ky in (0, 1, 2)]
                            for k, (ky, kx) in enumerate(taps):
                                idx = (ky * 3 + kx) * 2 + p
                                # xe col: x col kx-1 -> 0,1; 126+kx -> 2,3
                                xc = (kx - 1) if e == 0 else (kx + 2)
                                rhs = xe[gs, (ky + 1) * 4 + xc:
                                         (ky + 129) * 4 + xc:4]
                                mm(acc[:, 0:1, :],
                                   ws[gs, idx * 128:(idx + 1) * 128],
                                   rhs, k == 0, k == len(taps) - 1,
                                   (g * C, 0))
                            bcol = 2 * j + p
                            dst = ybufs[p][j][:, 1:H + 1,
                                              e * 127 + 1:e * 127 + 2]
                            if (e + j) % 2 == 0:
                                nc.scalar.activation(
                                    dst, acc[:, 0:1, :], IDENT,
                                    bias=b1_s[:, bcol:bcol + 1])
                            else:
                                nc.vector.tensor_scalar_add(
                                    dst, acc[:, 0:1, :],
                                    b1_s[:, bcol:bcol + 1])

            def stage2_pair(j):
                st = stg.tile([128, 8, 2 * W], BF16, name="st", tag="st")
                accs = {}
                for pp in (0, 1):
                    for qq in (0, 1):
                        for c in (0, 1):
                            accs[pp, qq, c] = psum_pool.tile(
                                [128, 4, W], F32, name="ps", tag="ps")
                for pp in (0, 1):
                    for k, (ky, kx) in enumerate(TAPS2):
                        blk = (ky * 3 + kx) * C
                        for c in (0, 1):
                            h0 = 4 * (2 * j + c)
                            cs = slice(c * C, (c + 1) * C)
                            for qq in (0, 1):
                                # fixed row group g=qq per accumulator (a
                                # PSUM bank must only ever be written from
                                # ONE quadrant: quadrants run concurrently
                                # and same-bank RMW from two would race).
                                p_in, q_in, dy, dx = _tap_decomp(
                                    pp, qq, ky, kx)
                                gs = slice(qq * C, (qq + 1) * C)
                                rhs = ybufs[p_in][q_in ^ qq][
                                    gs, h0 + dy + 1:h0 + dy + 5,
                                    dx + 1:dx + 1 + W]
                                mm(accs[pp, qq, c][cs, :, :],
                                   w2_s[gs, blk:blk + C],
                                   rhs, k == 0, k == 8, (qq * C, c * C))
                    for i, (qq, c) in enumerate(
                            ((0, 0), (0, 1), (1, 0), (1, 1))):
                        cs = slice(c * C, (c + 1) * C)
                        dst = st[cs, pp::2, qq::2]
                        if (i + pp) % 2 == 0:
                            nc.scalar.activation(dst, accs[pp, qq, c][cs],
                                                 IDENT, bias=b2_s[cs, 0:1])
                        else:
                            nc.vector.tensor_scalar_add(
                                dst, accs[pp, qq, c][cs], b2_s[cs, 0:1])
                if j < H // 8 - 1:
                    for c, eng in ((0, nc.sync), (1, nc.gpsimd)):
                        t = 2 * j + c
                        cs = slice(c * C, (c + 1) * C)
                        eng.dma_start(out=out_d[:, 8 * t:8 * t + 8, :],
                                      in_=st[cs, :, :])
                else:
                    for c in (0, 1):
                        t = 2 * j + c
                        cs = slice(c * C, (c + 1) * C)
                        for h, eng in ((0, nc.sync), (4, nc.gpsimd)):
                            eng.dma_start(
                                out=out_d[:, 8 * t + h:8 * t + h + 4, :],
                                in_=st[cs, h:h + 4, :])

            for sup in range(H // 16):
                stage1_super(sup)
            stage1_repair()
            for j in range(H // 8):
                stage2_pair(j)

    nc.compile()
    return nc


_MODULE_CACHE = {}


def _get_module():
    if "nc" not in _MODULE_CACHE:
        _MODULE_CACHE["nc"] = _build_module()
    return _MODULE_CACHE["nc"]


# ----------------------------------------------------------------------------
# Entry point
# ----------------------------------------------------------------------------

def prep_weight_map(pre_w, pre_b, post_w, post_b):
    w1, b1 = _build_stage1_weights(np.asarray(pre_w), np.asarray(pre_b))
    w2 = _build_stage2_weights(np.asarray(post_w))
    b2 = np.asarray(post_b, np.float32).reshape(C, 1)

    w1_half = np.transpose(w1, (3, 1, 2, 0, 4)).reshape(C, 18 * 128)
    w1_flat = np.ascontiguousarray(
        np.concatenate([w1_half, w1_half], axis=0)).astype(NP_BF16)
    # half-swapped m columns per idx block + half-swapped bias rows, for
    # the repair accs that write the partition-swapped Y duplicate.
    w1x = w1_flat.reshape(128, 18, 2, 64)[:, :, ::-1, :].reshape(128, -1)
    b1sw = np.roll(b1, 64, axis=0)
    b1_4 = np.concatenate([b1, b1sw], axis=1)          # [128, 4]
    return {
        "w1": w1_flat,
        "w1x": np.ascontiguousarray(w1x),
        "b1": np.ascontiguousarray(b1_4, np.float32),
        "w2": np.ascontiguousarray(w2).astype(NP_BF16),
        "b2": np.ascontiguousarray(np.vstack([b2, b2]), np.float32),
    }


def run(x, pre_w, pre_b, post_w, post_b, trace=False):
    x = np.asarray(x, np.float32)
    B = x.shape[0]
    assert B == N_CORES and x.shape == (B, C, H, W)

    wmap = prep_weight_map(pre_w, pre_b, post_w, post_b)
    x_bf = x.astype(NP_BF16)

    in_maps = []
    for b in range(B):
        xpad = np.zeros((C, HP, W), NP_BF16)
        xpad[:, 2:H + 2, :] = x_bf[b]
        xe = np.zeros((C, HP, 4), NP_BF16)
        xe[:, 2:H + 2, 0:2] = x_bf[b][:, :, 0:2]
        xe[:, 2:H + 2, 2:4] = x_bf[b][:, :, 126:128]
        in_maps.append({
            "xpad": np.ascontiguousarray(xpad.reshape(C, HP * W)),
            "xe": np.ascontiguousarray(xe.reshape(C, HP * 4)),
            **wmap,
        })

    nc = _get_module()
    res = run_bass_kernel_spmd(nc, in_maps, core_ids=list(range(N_CORES)),
                               trace=trace)
    out = np.stack([res.results[b]["out"].astype(np.float32)
                    for b in range(B)])
    return out, res


def kernel(x, pre_w, pre_b, post_w, post_b):
    out, _ = run(x, pre_w, pre_b, post_w, post_b)
    return out
